# revision 11
# baseline (speedup 1.0000x reference)
"""Bidirectional Mamba encoder layer on 8 TRN2 NeuronCores.

Sharding:
  K1 (scan): core = (batch b in 2) x (direction in {fwd,bwd}) x (state-half in 2).
     Each core runs the selective scan for 128 of the 256 state indices,
     full d_inner=256, full L=2048, producing a partial y^T (256, 2048) fp32.
     Main loop per state index s: dA = exp(expA[:,s]*dtneg) on ACT;
     data1 = dtuneg * (-B_s) broadcast; h = tensor_tensor_scan; w = h * C_s;
     y += I @ w accumulated in PSUM on the PE.
  Host: sums the state-half partial pairs (pure data routing + adds).
  K2 (rest): core = (b) x (seq quarter). Recomputes in-proj/conv slices,
     finalizes y (D term, silu gate), out-projects both directions into one
     PSUM accumulator, adds residual, LN1, FFN, LN2.
"""
import os
import numpy as np
import ml_dtypes
from contextlib import ExitStack

import concourse.bass as bass
import concourse.bacc as bacc
import concourse.mybir as mybir
import concourse.tile as tile
from concourse.bass_utils import run_bass_kernel_spmd

bf16 = ml_dtypes.bfloat16
BF = mybir.dt.bfloat16
F32 = mybir.dt.float32
AF = mybir.ActivationFunctionType
OP = mybir.AluOpType

B, L, DM = 2, 2048, 256
DS, DI, DTR, DCONV, DFF = 256, 256, 16, 2, 1024
EPS = 1e-5
TQ = L // 4  # K2 seq slice



# ----------------------------------------------------------------- K1 ----
def build_k1(ns=128, prep_reps=1, loop_reps=1):
    """prep_reps / loop_reps > 1 wrap the prep / state-loop sections in a
    hardware For_i — timing-only variants (production uses the defaults,
    which emit no loop instructions at all)."""
    from contextlib import nullcontext
    nc = bacc.Bacc("TRN2", target_bir_lowering=False, debug=False, num_devices=8)
    ins = {}
    def inp(name, shape, dt):
        ins[name] = nc.dram_tensor(name, shape, dt, kind="ExternalInput").ap()
    inp("xT", [DM, L], BF)
    inp("inwT_xi", [DM, DI], BF)
    inp("w0col", [DI, 1], F32)
    inp("w1col", [DI, 1], F32)
    inp("cbcol", [DI, 1], F32)
    inp("xprojT", [DI, 272], BF)      # rows: [B own (128) | C own (128) | dtlo (16)]
    inp("dtwT", [DTR, DI], BF)
    inp("dtb", [DI, 1], F32)
    inp("negA", [DI, 128], F32)
    inp("ident", [128, 128], BF)
    yT = nc.dram_tensor("yT", [DI, L], F32, kind="ExternalOutput").ap()
    scrB = nc.dram_tensor("scrB", [128, L], BF)
    scrC = nc.dram_tensor("scrC", [128, L], BF)

    with tile.TileContext(nc) as tc:
        with ExitStack() as ctx:
            const = ctx.enter_context(tc.tile_pool(name="const", bufs=1))
            prep_ctx = ExitStack()
            tmp2 = prep_ctx.enter_context(tc.tile_pool(name="tmp2", bufs=1))
            prep_loop = tc.For_i(0, prep_reps, 1) if prep_reps > 1 else nullcontext()
            prep_loop.__enter__()
            xt0 = tmp2.tile([128, L], BF, tag="xt0", name="xt0")
            xt1 = tmp2.tile([128, L], BF, tag="xt1", name="xt1")
            nc.sync.dma_start(out=xt0[:], in_=ins["xT"][0:128, :])
            nc.sync.dma_start(out=xt1[:], in_=ins["xT"][128:256, :])
            inw = []
            xpj = []
            w0c, w1c, cbc, dtbn = [], [], [], []
            for k in range(2):
                t = const.tile([128, DI], BF, tag=f"inwT{k}", name=f"inwT{k}")
                nc.sync.dma_start(out=t[:], in_=ins["inwT_xi"][k*128:(k+1)*128, :])
                inw.append(t)
                t = const.tile([128, 272], BF, tag=f"xprojT{k}", name=f"xprojT{k}")
                nc.sync.dma_start(out=t[:], in_=ins["xprojT"][k*128:(k+1)*128, :])
                xpj.append(t)
                for lst, nm in ((w0c, "w0col"), (w1c, "w1col"), (cbc, "cbcol"), (dtbn, "dtb")):
                    t = const.tile([128, 1], F32, tag=f"{nm}{k}", name=f"{nm}{k}")
                    nc.sync.dma_start(out=t[:], in_=ins[nm][k*128:(k+1)*128, :])
                    lst.append(t)
            dtwT = const.tile([DTR, DI], BF, tag="dtwT", name="dtwT")
            nc.sync.dma_start(out=dtwT[:], in_=ins["dtwT"][:])
            expA0 = const.tile([128, 128], F32, tag="expA0", name="expA0")
            expA1 = const.tile([128, 128], F32, tag="expA1", name="expA1")
            nc.sync.dma_start(out=expA0[:], in_=ins["negA"][0:128, :])
            nc.sync.dma_start(out=expA1[:], in_=ins["negA"][128:256, :])
            ident = const.tile([128, 128], BF, tag="ident", name="ident")
            nc.sync.dma_start(out=ident[:], in_=ins["ident"][:])

            xts = [xt0, xt1]
            NCH = 4  # 512-wide N chunks

            # ---- in-proj: xi^T = inwT.T @ xT  (two di m-chunks) ----
            xiT = []
            with tc.tile_pool(name="pp", bufs=1, space="PSUM") as pp, \
                 tc.tile_pool(name="tmp", bufs=1) as tmp:
                for m in range(2):
                    ps = pp.tile([128, L], F32, tag="ps", name="ps")
                    for k in range(2):
                        for c in range(NCH):
                            nc.tensor.matmul(ps[:, c*512:(c+1)*512],
                                             inw[k][:, m*128:(m+1)*128],
                                             xts[k][:, c*512:(c+1)*512],
                                             start=(k == 0), stop=(k == 1))
                    xit = tmp2.tile([128, L + 1], BF, tag=f"xiT{m}", name=f"xiT{m}")
                    nc.vector.memset(xit[:, 0:1], 0.0)
                    nc.vector.tensor_copy(xit[:, 1:L+1], ps[:])
                    xiT.append(xit)

                # ---- conv + silu -> xcT ----
                xcT = []
                for m in range(2):
                    q = tmp.tile([128, L], F32, tag=f"q{m}", name=f"q{m}")
                    nc.vector.tensor_scalar(q[:], xiT[m][:, 1:L+1],
                                            scalar1=w1c[m][:],
                                            scalar2=cbc[m][:],
                                            op0=OP.mult, op1=OP.add)
                    pre = tmp.tile([128, L], F32, tag=f"pre{m}", name=f"pre{m}")
                    nc.vector.scalar_tensor_tensor(pre[:], xiT[m][:, 0:L],
                                                   w0c[m][:], q[:],
                                                   op0=OP.mult, op1=OP.add)
                    xc = const.tile([128, L], BF, tag=f"xcT{m}", name=f"xcT{m}")
                    nc.scalar.activation(xc[:], pre[:], AF.Silu)
                    xcT.append(xc)

                # ---- xproj: [-B | C | dtlo]^T = xprojT.T @ xcT ----
                BnegT = const.tile([128, L], BF, tag="BnegT", name="BnegT")
                CT = const.tile([128, L], BF, tag="CT", name="CT")
                dtloT = const.tile([DTR, L], BF, tag="dtloT", name="dtloT")
                for mi, (mlo, msz, dst) in enumerate([(0, 128, BnegT), (128, 128, CT), (256, 16, dtloT)]):
                    ps2 = pp.tile([128, L], F32, tag="ps", name="ps")
                    for k in range(2):
                        for c in range(NCH):
                            nc.tensor.matmul(ps2[0:msz, c*512:(c+1)*512],
                                             xpj[k][:, mlo:mlo+msz],
                                             xcT[k][:, c*512:(c+1)*512],
                                             start=(k == 0), stop=(k == 1))
                    nc.vector.tensor_copy(dst[:], ps2[0:msz, :])
                nc.sync.dma_start(out=scrB.ap(), in_=BnegT[:])
                nc.sync.dma_start(out=scrC.ap(), in_=CT[:])

                # ---- dtneg^T = ln(sigmoid(-(dtw@dtlo + dt_b))) = -dt ----
                dtnegT = []
                dtunegT = []
                for m in range(2):
                    ps3 = pp.tile([128, L], F32, tag="ps", name="ps")
                    for c in range(NCH):
                        nc.tensor.matmul(ps3[:, c*512:(c+1)*512],
                                         dtwT[0:DTR, m*128:(m+1)*128],
                                         dtloT[0:DTR, c*512:(c+1)*512],
                                         start=True, stop=True)
                    sg = tmp.tile([128, L], F32, tag=f"sgx{m}", name=f"sgx{m}")
                    nc.scalar.activation(sg[:], ps3[:], AF.Sigmoid, scale=-1.0,
                                         bias=dtbn[m][:])
                    dtn = const.tile([128, L], BF, tag=f"dtn{m}", name=f"dtn{m}")
                    nc.scalar.activation(dtn[:], sg[:], AF.Ln)
                    dtnegT.append(dtn)
                    dtu = const.tile([128, L], BF, tag=f"dtu{m}", name=f"dtu{m}")
                    nc.vector.tensor_tensor(dtu[:], dtn[:], xcT[m][:], OP.mult)
                    dtunegT.append(dtu)

            prep_loop.__exit__(None, None, None)
            # ---- main scan loop over own 128 state indices ----
            prep_ctx.close()
            expAs = [expA0, expA1]
            LL = 2 * L + 1   # [m0 | zero boundary | m1]
            dAcat, d1cat = [], []
            for bi in range(2):
                t = const.tile([128, LL], F32, tag=f"dAcat{bi}", name=f"dAcat{bi}")
                nc.vector.memset(t[:, L:L+1], 0.0)
                dAcat.append(t)
                t = const.tile([128, LL], BF, tag=f"d1cat{bi}", name=f"d1cat{bi}")
                nc.vector.memset(t[:, L:L+1], 0.0)
                d1cat.append(t)
            with tc.tile_pool(name="lp", bufs=3) as lp, \
                 tc.tile_pool(name="yps", bufs=1, space="PSUM") as yps:
                yp = [yps.tile([128, L], F32, tag="y0", name="y0"),
                      yps.tile([128, L], F32, tag="y1", name="y1")]
                main_loop = tc.For_i(0, loop_reps, 1) if loop_reps > 1 else nullcontext()
                main_loop.__enter__()
                for s in range(ns):
                    b_bc = lp.tile([128, L], BF, tag="b_bc", name="b_bc")
                    c_bc = lp.tile([128, L], BF, tag="c_bc", name="c_bc")
                    nc.sync.dma_start(out=b_bc[:], in_=bass.AP(scrB, scrB.ap()[s:s+1, :].offset, [[0, 128], [1, L]]))
                    nc.sync.dma_start(out=c_bc[:], in_=bass.AP(scrC, scrC.ap()[s:s+1, :].offset, [[0, 128], [1, L]]))
                    da = dAcat[s % 2]
                    d1 = d1cat[s % 2]
                    for m in range(2):
                        off = m * (L + 1)
                        nc.scalar.activation(da[:, off:off+L], dtnegT[m][:], AF.Exp,
                                             scale=expAs[m][:, s:s+1])
                        nc.vector.tensor_tensor(d1[:, off:off+L], dtunegT[m][:], b_bc[:], OP.mult)
                    h = lp.tile([128, LL], BF, tag="h", name="h")
                    nc.vector.tensor_tensor_scan(h[:], da[:], d1[:], 0.0, OP.mult, OP.add)
                    for m in range(2):
                        off = m * (L + 1)
                        w = lp.tile([128, L], BF, tag=f"w{m}", name=f"w{m}", bufs=2)
                        nc.vector.tensor_tensor(w[:], h[:, off:off+L], c_bc[:], OP.mult)
                        for c in range(NCH):
                            nc.tensor.matmul(yp[m][:, c*512:(c+1)*512], ident[:],
                                             w[:, c*512:(c+1)*512],
                                             start=(s == 0), stop=(s == ns - 1))
                for m in range(2):
                    ysb = const.tile([128, L], F32, tag=f"ysb{m}", name=f"ysb{m}")
                    nc.vector.tensor_copy(ysb[:], yp[m][:])
                    nc.sync.dma_start(out=yT[m*128:(m+1)*128, :], in_=ysb[:])
                main_loop.__exit__(None, None, None)
    nc.compile()
    return nc


# ----------------------------------------------------------------- K2 ----
def build_k2(body_reps=1):
    from contextlib import nullcontext
    TE = TQ + 2  # extended slice with boundary cols
    nc = bacc.Bacc("TRN2", target_bir_lowering=False, debug=False, num_devices=8)
    ins = {}
    def inp(name, shape, dt):
        ins[name] = nc.dram_tensor(name, shape, dt, kind="ExternalInput").ap()
    # per-direction inputs (f suffix fwd, b suffix bwd; bwd already in fwd time)
    for d in ("f", "b"):
        inp(f"yT_{d}", [DI, TQ], F32)
        inp(f"xTe_{d}", [DM, TE], BF)
        inp(f"inwT_{d}", [DM, 2 * DI], BF)
        inp(f"w0col_{d}", [DI, 1], F32)
        inp(f"w1col_{d}", [DI, 1], F32)
        inp(f"cbcol_{d}", [DI, 1], F32)
        inp(f"Dcol_{d}", [DI, 1], F32)
        inp(f"outwT_{d}", [DI, DM], BF)
    inp("x_t", [TQ, DM], F32)
    inp("g1bc", [128, DM], F32)
    inp("b1bc", [128, DM], F32)
    inp("g2bc", [128, DM], F32)
    inp("b2bc", [128, DM], F32)
    inp("w1T", [DM, DFF], BF)
    inp("b1col", [DFF, 1], F32)
    inp("w2T", [DFF, DM], BF)
    inp("b2col", [DM, 1], F32)
    inp("ident", [128, 128], BF)
    out = nc.dram_tensor("out", [TQ, DM], F32, kind="ExternalOutput").ap()

    with tile.TileContext(nc) as tc:
        with ExitStack() as ctx:
            sb = ctx.enter_context(tc.tile_pool(name="sb", bufs=1))
            pp = ctx.enter_context(tc.tile_pool(name="pp", bufs=1, space="PSUM"))
            ppa = ctx.enter_context(tc.tile_pool(name="ppa", bufs=1, space="PSUM"))

            body_loop = tc.For_i(0, body_reps, 1) if body_reps > 1 else nullcontext()
            body_loop.__enter__()
            ident = sb.tile([128, 128], BF, tag="ident", name="ident")
            nc.sync.dma_start(out=ident[:], in_=ins["ident"][:])
            epsc = sb.tile([128, 1], F32, tag="epsc", name="epsc")
            nc.vector.memset(epsc[:], EPS)

            # ---------- per-direction: conv, finalize y, out-proj ----------
            MPS = [ppa.tile([128, TQ], F32, tag="mps0", name="mps0"),
                   ppa.tile([128, TQ], F32, tag="mps1", name="mps1")]
            first_mm = [True, True]
            for di_, d in enumerate(("f", "b")):
                xte = [sb.tile([128, TE], BF, tag=f"xte0{d}", name=f"xte0{d}"),
                       sb.tile([128, TE], BF, tag=f"xte1{d}", name=f"xte1{d}")]
                nc.sync.dma_start(out=xte[0][:], in_=ins[f"xTe_{d}"][0:128, :])
                nc.sync.dma_start(out=xte[1][:], in_=ins[f"xTe_{d}"][128:256, :])
                inw = []
                for k in range(2):
                    t = sb.tile([128, 2 * DI], BF, tag=f"inwT{k}{d}", name=f"inwT{k}{d}")
                    nc.sync.dma_start(out=t[:], in_=ins[f"inwT_{d}"][k*128:(k+1)*128, :])
                    inw.append(t)
                cols = {}
                for nm in ("w0col", "w1col", "cbcol", "Dcol"):
                    pair = []
                    for k in range(2):
                        t = sb.tile([128, 1], F32, tag=f"{nm}{k}{d}", name=f"{nm}{k}{d}")
                        nc.sync.dma_start(out=t[:], in_=ins[f"{nm}_{d}"][k*128:(k+1)*128, :])
                        pair.append(t)
                    cols[nm] = pair
                # in-proj on extended slice: xi (m 0..1) and z (m 2..3)
                xzT = []
                for m in range(4):
                    ps = pp.tile([128, TE], F32, tag="ps", name="ps")
                    for k in range(2):
                        nc.tensor.matmul(ps[:, 0:512],
                                         inw[k][:, m*128:(m+1)*128],
                                         xte[k][:, 0:512], start=(k == 0), stop=(k == 1))
                        nc.tensor.matmul(ps[:, 512:TE],
                                         inw[k][:, m*128:(m+1)*128],
                                         xte[k][:, 512:TE], start=(k == 0), stop=(k == 1))
                    t = sb.tile([128, TE], BF, tag=f"xzT{m}{d}", name=f"xzT{m}{d}")
                    nc.vector.tensor_copy(t[:], ps[:])
                    xzT.append(t)
                # conv (fwd: prev=j, cur=j+1 ; bwd: prev=j+2, cur=j+1)
                off_prev = 0 if d == "f" else 2
                ytil = []
                for m in range(2):
                    q = sb.tile([128, TQ], F32, tag=f"q{m}{d}", name=f"q{m}{d}")
                    nc.vector.tensor_scalar(q[:], xzT[m][:, 1:TQ+1],
                                            scalar1=cols["w1col"][m][:],
                                            scalar2=cols["cbcol"][m][:],
                                            op0=OP.mult, op1=OP.add)
                    pre = sb.tile([128, TQ], F32, tag=f"pre{m}{d}", name=f"pre{m}{d}")
                    nc.vector.scalar_tensor_tensor(pre[:], xzT[m][:, off_prev:off_prev+TQ],
                                                   cols["w0col"][m][:], q[:],
                                                   op0=OP.mult, op1=OP.add)
                    xc = sb.tile([128, TQ], F32, tag=f"xc{m}{d}", name=f"xc{m}{d}")
                    nc.scalar.activation(xc[:], pre[:], AF.Silu)
                    # y_full + xc*D
                    yt = sb.tile([128, TQ], F32, tag=f"yt{m}{d}", name=f"yt{m}{d}")
                    nc.sync.dma_start(out=yt[:], in_=ins[f"yT_{d}"][m*128:(m+1)*128, :])
                    t1 = sb.tile([128, TQ], F32, tag=f"t1{m}{d}", name=f"t1{m}{d}")
                    nc.vector.scalar_tensor_tensor(t1[:], xc[:],
                                                   cols["Dcol"][m][:], yt[:],
                                                   op0=OP.mult, op1=OP.add)
                    # silu(z)
                    zt = xzT[2 + m]
                    sil = sb.tile([128, TQ], F32, tag=f"sil{m}{d}", name=f"sil{m}{d}")
                    nc.scalar.activation(sil[:], zt[:, 1:TQ+1], AF.Silu)
                    yy = sb.tile([128, TQ], BF, tag=f"yy{m}{d}", name=f"yy{m}{d}")
                    nc.vector.tensor_tensor(yy[:], t1[:], sil[:], OP.mult)
                    ytil.append(yy)
                # out-proj accumulate into MPS
                outw = []
                for k in range(2):
                    t = sb.tile([128, DM], BF, tag=f"outwT{k}{d}", name=f"outwT{k}{d}")
                    nc.sync.dma_start(out=t[:], in_=ins[f"outwT_{d}"][k*128:(k+1)*128, :])
                    outw.append(t)
                for m in range(2):
                    for k in range(2):
                        nc.tensor.matmul(MPS[m][:, :],
                                         outw[k][:, m*128:(m+1)*128],
                                         ytil[k][:, :],
                                         start=first_mm[m], stop=(di_ == 1 and k == 1))
                        first_mm[m] = False

            # ---------- M^T -> t-major, residual, LN1 ----------
            mT = [sb.tile([128, TQ], BF, tag="mT0", name="mT0"), sb.tile([128, TQ], BF, tag="mT1", name="mT1")]
            for m in range(2):
                nc.vector.tensor_copy(mT[m][:], MPS[m][:])

            def transpose_to(src_tiles, nrow_tiles, ncol_tiles, tagp, dtype=F32):
                # src: list of (128, ncol_tiles*128) bf16 tiles (row-major chunks)
                outs = []
                for r in range(ncol_tiles):
                    dst = sb.tile([128, nrow_tiles * 128], dtype, tag=f"{tagp}{r}", name=f"{tagp}{r}")
                    for cidx in range(nrow_tiles):
                        pt = pp.tile([128, 128], BF, tag="tps", name="tps")
                        nc.tensor.transpose(pt[:], src_tiles[cidx][:, r*128:(r+1)*128], ident[:])
                        nc.vector.tensor_copy(dst[:, cidx*128:(cidx+1)*128], pt[:])
                    outs.append(dst)
                return outs

            m_t = transpose_to(mT, 2, TQ // 128, "m_t")  # 4 tiles (128t, 256dm) f32

            xr = sb.tile([128, DM], F32, tag="xr", name="xr")
            LNT = []
            g1 = sb.tile([128, DM], F32, tag="g1bc", name="g1bc")
            b1 = sb.tile([128, DM], F32, tag="b1bc", name="b1bc")
            g2 = sb.tile([128, DM], F32, tag="g2bc", name="g2bc")
            b2 = sb.tile([128, DM], F32, tag="b2bc", name="b2bc")
            for t, nm in ((g1, "g1bc"), (b1, "b1bc"), (g2, "g2bc"), (b2, "b2bc")):
                nc.sync.dma_start(out=t[:], in_=ins[nm][:])

            def layernorm(xin, g, b, tag):
                mean = sb.tile([128, 1], F32, tag=f"mn{tag}", name=f"mn{tag}")
                nc.vector.tensor_reduce(mean[:], xin[:], mybir.AxisListType.X, OP.add)
                nc.vector.tensor_scalar(mean[:], mean[:], scalar1=-1.0/DM, scalar2=None, op0=OP.mult)
                xm = sb.tile([128, DM], F32, tag=f"xm{tag}", name=f"xm{tag}")
                nc.vector.tensor_scalar(xm[:], xin[:], scalar1=mean[:], scalar2=None, op0=OP.add)
                sq = sb.tile([128, DM], F32, tag=f"sq{tag}", name=f"sq{tag}")
                nc.scalar.activation(sq[:], xm[:], AF.Square)
                var = sb.tile([128, 1], F32, tag=f"vr{tag}", name=f"vr{tag}")
                nc.vector.tensor_reduce(var[:], sq[:], mybir.AxisListType.X, OP.add)
                sd = sb.tile([128, 1], F32, tag=f"sd{tag}", name=f"sd{tag}")
                nc.scalar.activation(sd[:], var[:], AF.Sqrt, scale=1.0/DM, bias=epsc[:])
                rs = sb.tile([128, 1], F32, tag=f"rs{tag}", name=f"rs{tag}")
                nc.vector.reciprocal(rs[:], sd[:])
                xn = sb.tile([128, DM], F32, tag=f"xn{tag}", name=f"xn{tag}")
                nc.vector.tensor_scalar(xn[:], xm[:], scalar1=rs[:], scalar2=None, op0=OP.mult)
                xg = sb.tile([128, DM], F32, tag=f"xg{tag}", name=f"xg{tag}")
                nc.vector.tensor_tensor(xg[:], xn[:], g[:], OP.mult)
                nc.vector.tensor_tensor(xg[:], xg[:], b[:], OP.add)
                return xg

            xn_t = []   # LN1 outputs, t-major f32, 4 tiles (128, 256)
            for r in range(TQ // 128):
                nc.sync.dma_start(out=xr[:], in_=ins["x_t"][r*128:(r+1)*128, :])
                ladd = sb.tile([128, DM], F32, tag=f"ladd{r}", name=f"ladd{r}")
                nc.vector.tensor_tensor(ladd[:], xr[:], m_t[r][:], OP.add)
                xn = layernorm(ladd, g1, b1, f"l1_{r}")
                xn_t.append(xn)

            # ---------- transpose xn -> dm-major bf16 ----------
            xn_bf = []
            for r in range(TQ // 128):
                t = sb.tile([128, DM], BF, tag=f"xnbf{r}", name=f"xnbf{r}")
                nc.vector.tensor_copy(t[:], xn_t[r][:])
                xn_bf.append(t)
            xnT = transpose_to(xn_bf, TQ // 128, 2, "xnT", dtype=BF)  # 2 tiles (128dm, 512t)

            # ---------- FFN ----------
            w1t = []
            for k in range(2):
                t = sb.tile([128, DFF], BF, tag=f"w1T{k}", name=f"w1T{k}")
                nc.sync.dma_start(out=t[:], in_=ins["w1T"][k*128:(k+1)*128, :])
                w1t.append(t)
            b1cs = []
            for k in range(DFF // 128):
                t = sb.tile([128, 1], F32, tag=f"b1col{k}", name=f"b1col{k}")
                nc.sync.dma_start(out=t[:], in_=ins["b1col"][k*128:(k+1)*128, :])
                b1cs.append(t)
            w2t = []
            for k in range(DFF // 128):
                t = sb.tile([128, DM], BF, tag=f"w2T{k}", name=f"w2T{k}")
                nc.sync.dma_start(out=t[:], in_=ins["w2T"][k*128:(k+1)*128, :])
                w2t.append(t)
            b2cs = []
            for k in range(2):
                t = sb.tile([128, 1], F32, tag=f"b2col{k}", name=f"b2col{k}")
                nc.sync.dma_start(out=t[:], in_=ins["b2col"][k*128:(k+1)*128, :])
                b2cs.append(t)

            h1 = []
            for m in range(DFF // 128):
                ps = pp.tile([128, TQ], F32, tag="ps2", name="ps2")
                for k in range(2):
                    nc.tensor.matmul(ps[:], w1t[k][:, m*128:(m+1)*128],
                                     xnT[k][:], start=(k == 0), stop=(k == 1))
                t = sb.tile([128, TQ], BF, tag=f"h1{m}", name=f"h1{m}")
                nc.scalar.activation(t[:], ps[:], AF.Relu, bias=b1cs[m][:])
                h1.append(t)
            f2 = []
            for m in range(2):
                ps = pp.tile([128, TQ], F32, tag="ps2", name="ps2")
                for k in range(DFF // 128):
                    nc.tensor.matmul(ps[:], w2t[k][:, m*128:(m+1)*128],
                                     h1[k][:], start=(k == 0), stop=(k == DFF//128 - 1))
                t = sb.tile([128, TQ], BF, tag=f"f2{m}", name=f"f2{m}")
                nc.vector.tensor_scalar(t[:], ps[:], scalar1=b2cs[m][:],
                                        scalar2=None, op0=OP.add)
                f2.append(t)
            f2_t = transpose_to(f2, 2, TQ // 128, "f2t")  # t-major f32

            for r in range(TQ // 128):
                sm = sb.tile([128, DM], F32, tag=f"sm{r}", name=f"sm{r}")
                nc.vector.tensor_tensor(sm[:], xn_t[r][:], f2_t[r][:], OP.add)
                o = layernorm(sm, g2, b2, f"l2_{r}")
                nc.sync.dma_start(out=out[r*128:(r+1)*128, :], in_=o[:])
            body_loop.__exit__(None, None, None)
    nc.compile()
    return nc


# ------------------------------------------------------------- host glue ----
_cache = {}
_debug = {}


def _runners():
    if "k1" not in _cache:
        _cache["k1"] = build_k1()
        _cache["k2"] = build_k2()
    return _cache["k1"], _cache["k2"]


def kernel(**inputs):
    np32 = lambda a: np.asarray(a, np.float32)
    x = np32(inputs["x"])  # (2, 2048, 256)
    k1, k2 = _runners()

    prm = {}
    for d, p in (("f", "mf"), ("b", "mb")):
        prm[d] = {k: np32(inputs[f"{p}_{k}"]) for k in
                  ["in_w", "conv_w", "conv_b", "xproj_w", "dt_w", "dt_b", "Alog", "D", "out_w"]}

    ident = np.eye(128).astype(bf16)
    # ---- K1 inputs per core: core = b*4 + dir*2 + shalf ----
    in_maps1 = []
    for b in range(B):
        for d in ("f", "b"):
            w = prm[d]
            xb = x[b]
            if d == "b":
                xb = xb[::-1]
            xT = np.ascontiguousarray(xb.T).astype(bf16)
            inwT_xi = np.ascontiguousarray(w["in_w"][:DI, :].T).astype(bf16)
            negA_full = np.exp(w["Alog"]).astype(np.float32)  # positive; pairs with dtneg
            for sh in range(2):
                rows = np.concatenate([
                    -w["xproj_w"][DTR + sh*128: DTR + sh*128 + 128, :],   # -B own
                    w["xproj_w"][DTR + DS + sh*128: DTR + DS + sh*128 + 128, :],  # C own
                    w["xproj_w"][0:DTR, :],                                # dtlo
                ], axis=0)  # (272, 256)
                in_maps1.append({
                    "xT": xT,
                    "inwT_xi": inwT_xi,
                    "w0col": np.ascontiguousarray(w["conv_w"][:, 0:1]),
                    "w1col": np.ascontiguousarray(w["conv_w"][:, 1:2]),
                    "cbcol": np.ascontiguousarray(w["conv_b"][:, None]),
                    "xprojT": np.ascontiguousarray(rows.T).astype(bf16),
                    "dtwT": np.ascontiguousarray(w["dt_w"].T).astype(bf16),
                    "dtb": np.ascontiguousarray(-w["dt_b"][:, None]),
                    "negA": np.ascontiguousarray(negA_full[:, sh*128:(sh+1)*128]),
                    "ident": ident,
                })
    res1 = run_bass_kernel_spmd(k1, in_maps1, list(range(8))).results
    _debug['res1'] = res1; _debug['in_maps1'] = in_maps1
    # combine shalf pairs; unflip bwd
    yT = {}
    for b in range(B):
        for j, d in enumerate(("f", "b")):
            i0 = b*4 + j*2
            yt = res1[i0]["yT"] + res1[i0+1]["yT"]  # (256, 2048) f32
            if d == "b":
                yt = yt[:, ::-1]
            yT[(b, d)] = yt

    # ---- K2 inputs per core: core = b*4 + tq ----
    TE = TQ + 2
    in_maps2 = []
    for b in range(B):
        xb = x[b]
        xbT = xb.T  # (256, 2048)
        for tq in range(4):
            m = {"x_t": np.ascontiguousarray(xb[tq*TQ:(tq+1)*TQ, :]),
                 "ident": ident,
                 "g1bc": np.broadcast_to(np32(inputs["n1_g"]), (128, DM)).copy(),
                 "b1bc": np.broadcast_to(np32(inputs["n1_b"]), (128, DM)).copy(),
                 "g2bc": np.broadcast_to(np32(inputs["n2_g"]), (128, DM)).copy(),
                 "b2bc": np.broadcast_to(np32(inputs["n2_b"]), (128, DM)).copy(),
                 "w1T": np.ascontiguousarray(np32(inputs["ffn_w1"]).T).astype(bf16),
                 "b1col": np.ascontiguousarray(np32(inputs["ffn_b1"])[:, None]),
                 "w2T": np.ascontiguousarray(np32(inputs["ffn_w2"]).T).astype(bf16),
                 "b2col": np.ascontiguousarray(np32(inputs["ffn_b2"])[:, None]),
                 }
            for d in ("f", "b"):
                w = prm[d]
                # everything in K2 is forward-time; the bwd direction's conv is
                # anti-causal there (off_prev=2 in build_k2)
                lo = tq * TQ
                ext = np.zeros((DM, TE), np.float32)
                s0, s1 = lo - 1, lo + TQ + 1
                c0, c1 = max(s0, 0), min(s1, L)
                ext[:, c0 - s0: c0 - s0 + (c1 - c0)] = xbT[:, c0:c1]
                ytd = yT[(b, d)][:, lo:lo+TQ]
                m[f"xTe_{d}"] = np.ascontiguousarray(ext).astype(bf16)
                m[f"yT_{d}"] = np.ascontiguousarray(ytd)
                m[f"inwT_{d}"] = np.ascontiguousarray(w["in_w"].T).astype(bf16)
                m[f"w0col_{d}"] = np.ascontiguousarray(w["conv_w"][:, 0:1])
                m[f"w1col_{d}"] = np.ascontiguousarray(w["conv_w"][:, 1:2])
                m[f"cbcol_{d}"] = np.ascontiguousarray(w["conv_b"][:, None])
                m[f"Dcol_{d}"] = np.ascontiguousarray(w["D"][:, None])
                m[f"outwT_{d}"] = np.ascontiguousarray(w["out_w"].T).astype(bf16)
            in_maps2.append(m)
    res2 = run_bass_kernel_spmd(k2, in_maps2, list(range(8))).results
    _debug['res2'] = res2; _debug['in_maps2'] = in_maps2; _debug['yT'] = yT

    out = np.zeros((B, L, DM), np.float32)
    for b in range(B):
        for tq in range(4):
            out[b, tq*TQ:(tq+1)*TQ, :] = res2[b*4 + tq]["out"]

    return out

